# revision 9
# baseline (speedup 1.0000x reference)
"""MoE routing kernel for Trainium2 (8 NeuronCores, batch-parallel).

Problem: nn_MoE_47278999994656.
  x [8, 256, 80, 80] f32 + gate Linear(256->5) + 5 experts
  (residual conv1x1 on each 128-ch half, gated by a sigmoid transform),
  top-1 masked-softmax gate => weights are EXACTLY one-hot, so
  out[b] = expert_{argmax_e logits[b,e]}(x[b]).

Sharding: data-parallel over batch, core i computes batch item i.

Restructured dataflow (vs the straightforward D->H->A->combine):
  reference: D_h = (I+W_h) x_h + b_h ; s_h = sigmoid(Wt2 relu(Wt1 D_h + bt1) + bt2)
             out = s0*D0 + s1*D1   (s per-column scalars)
  Per-column scaling commutes with channel mixing:
     s_h * D_h = (I+W_h) (s_h * x_h) + b_h (x) s_h
  and the sigmoid path folds to  h = [Wt1(I+W_h)] x_h + (Wt1 b_h + bt1).
  The b_h (x) s_h rank-1 term rides the same PSUM accumulation as two
  contract-1 matmuls whose rhs is a (replicated) sigmoid row.
  This deletes the D-layer evictions and the 3-op combine entirely.

Per 1024-col chunk-pair: H matmuls (from x) -> paired relu -> A matmuls
(replicated Wt2, 4 psum banks) -> ONE 4-bank sigmoid -> y = x*s ->
final (I+W_h)^T matmuls + rank-1 bias -> paired evict -> DMA out.

Performance notes (measured on HW):
  - PE runs 512-col bf16 matmuls at 216ns only after ~3.2us of GAPLESS
    execution (else 427+), and any dep-wait breaks the streak. So:
    phase 1 keeps the PE on a dependency-free junk-matmul stream (the
    gate pooling runs on the otherwise-idle DVE as chunks arrive), and
    phase 2 is software-pipelined with deep skew (A one pair behind H,
    F three behind) so every matmul's deps are long since satisfied.
  - One DMA ring sustains only ~130 GB/s; x is spread over the 3
    dispatch engines (sync/scalar/gpsimd).
  - gpsimd (Pool) is too slow for bulk elementwise work (~2ns/elem);
    it only dispatches DMAs here.
"""

import numpy as np

import concourse.bacc as bacc_mod
import concourse.bass as bass
import concourse.mybir as mybir
import concourse.tile as tile
from concourse.bass_utils import run_bass_kernel_spmd

B, C, H, W = 8, 256, 80, 80
HW = H * W          # 6400
HALF = 128
QUARTER = 64
E = 5
NCORES = 8

# phase-2: 6 pairs of 512-col chunks + one 256-col single
PAIRS = [(p * 1024, 512, 512) for p in range(6)] + [(6144, 256, 0)]
NP = len(PAIRS)
FSKEW = 3           # F matmuls run this many pair-iterations behind H

# input x DMA chunks (columns), round-robin over 3 rings; small last chunk
XCH = [(i * 768, 768) for i in range(8)] + [(6144, 256)]

# U free-dim layout (per expert, partition dim = 128):
#   [0:128)    F_rgb = (I + Wrgb)^T           [c, o]
#   [128:256)  F_tir = (I + Wtir)^T           [c, o]
#   [256:320)  Wh_rgb^T = (Wt1 @ F_rgb^T)^T   [c, m]  (m = 64)
#   [320:384)  Wh_tir^T                        [c, m]
#   [384]      wt2 stacked: row r = Wt2[r % 64] (expanded to 128 cols on dev)
UF = 385
U_F0 = 0
U_F1 = 128
U_WH0 = 256
U_WH1 = 320
U_WT2 = 384
USEL_F = 512

# bias table columns: 0 = relu bias (Wt1 b_h + bt1, halves stacked),
# 1 = bt2 (replicated); cols 2,3 unused
NBIAS = 4

F32 = mybir.dt.float32
BF16 = mybir.dt.bfloat16


def build_nc() -> bass.Bass:
    nc = bacc_mod.Bacc()

    x_d = nc.dram_tensor("x", [HALF, 2, HW], BF16, kind="ExternalInput")
    u_d = nc.dram_tensor("u", [HALF, E, UF], BF16, kind="ExternalInput")
    bias_d = nc.dram_tensor("bias", [HALF, E, NBIAS], F32, kind="ExternalInput")
    btr_d = nc.dram_tensor("btr", [1, E, 2, HALF], BF16, kind="ExternalInput")
    wg_d = nc.dram_tensor("wg", [HALF, 2, E], BF16, kind="ExternalInput")
    bg_d = nc.dram_tensor("bg", [1, E], F32, kind="ExternalInput")
    out_d = nc.dram_tensor("out", [HALF, HW], BF16, kind="ExternalOutput")

    with tile.TileContext(nc) as tc:
        with (
            tc.tile_pool(name="big", bufs=1) as big,
            tc.tile_pool(name="const", bufs=1) as const,
            tc.tile_pool(name="small", bufs=1) as small,
            tc.tile_pool(name="sb", bufs=4) as sb,
            tc.tile_pool(name="sel", bufs=1) as sel,
            tc.tile_pool(name="ps", bufs=1, space="PSUM") as ps,
        ):
            # ---- persistent SBUF ----
            xb = big.tile([HALF, 2, HW], BF16)       # 25.6 KB/part
            osb = big.tile([HALF, HW], BF16)         # 12.8 KB/part
            u_all = const.tile([HALF, E, UF], BF16)
            bias_all = const.tile([HALF, E, NBIAS], F32)
            btr_all = const.tile([1, E, 2, HALF], BF16)
            wg = const.tile([HALF, 2, E], BF16)
            bgx = const.tile([1, E], F32)
            usel = const.tile([HALF, USEL_F], BF16)
            bsel = const.tile([HALF, NBIAS], F32)
            btr = const.tile([1, 2, HALF], BF16)
            ones1024 = const.tile([HALF, 1024], BF16)
            ones128 = const.tile([HALF, HALF], BF16)
            ones1 = const.tile([1, HALF], F32)
            pp = const.tile([HALF, 2, len(XCH)], F32)
            ph = const.tile([HALF, 2], BF16)
            t32a = small.tile([32, 32], F32)
            t32b = small.tile([32, 32], F32)

            # ---- DMA dispatch: x over 3 rings; weights behind x on their
            # rings (nothing needs them before ~10us) ----
            rings = [nc.sync, nc.scalar, nc.gpsimd]
            for k, (o, n) in enumerate(XCH):
                rings[k % 3].dma_start(
                    out=xb[:, :, o : o + n], in_=x_d[:, :, o : o + n]
                )
            nc.gpsimd.dma_start(out=u_all[:], in_=u_d[:])
            nc.sync.dma_start(out=bias_all[:], in_=bias_d[:])
            nc.sync.dma_start(out=btr_all[:], in_=btr_d[:])
            nc.sync.dma_start(out=wg[:], in_=wg_d[:])
            nc.sync.dma_start(out=bgx[:], in_=bg_d[:])

            # DVE: constants first (junk matmuls depend on the ones tiles)
            nc.vector.memset(ones128, 1.0)
            nc.vector.memset(ones1024, 1.0)
            nc.vector.memset(t32a, 0.0)
            nc.vector.memset(ones1, 1.0)

            # dependency-free junk matmuls keep the PE p-state ramped
            def junk(rhs=None, nf=512):
                jp = ps.tile([HALF, 2, 512], F32, tag="hps2", name="jp")
                nc.tensor.matmul(
                    jp[:, 0, 0:nf],
                    lhsT=ones128,
                    rhs=rhs if rhs is not None else ones1024[:, 0:512],
                )

            # ---- phase 1: DVE pools x per chunk as it arrives ----
            for k, (o, n) in enumerate(XCH):
                for h in range(2):
                    nc.vector.reduce_sum(
                        pp[:, h, k : k + 1], xb[:, h, o : o + n],
                        axis=mybir.AxisListType.X,
                    )
                junk()
                junk()
                junk()
                junk()
            with nc.allow_low_precision(reason="pooled sums ~1e2, bf16 noise 80x under gate margin"):
                for h in range(2):
                    nc.vector.reduce_sum(
                        ph[:, h : h + 1], pp[:, h, :], axis=mybir.AxisListType.X
                    )
            junk()
            junk()
            # logits via two contract-128, 1-col matmuls
            yg = ps.tile([E, 512], F32, tag="outps", name="yg")
            for h in range(2):
                nc.tensor.matmul(
                    yg[0:E, 0:1], lhsT=wg[:, h, :], rhs=ph[:, h : h + 1],
                    start=(h == 0), stop=(h == 1),
                )
            junk()
            nc.vector.tensor_copy(t32a[0:E, 0:1], yg[0:E, 0:1])
            junk()
            nc.vector.transpose(t32b, t32a)
            lrow = small.tile([1, E], F32)
            nc.vector.tensor_add(lrow, t32b[0:1, 0:E], bgx[0:1, :])
            junk()
            lmax = small.tile([1, 1], F32)
            nc.vector.reduce_max(lmax, lrow, axis=mybir.AxisListType.X)
            mrow = small.tile([1, E], F32)
            nc.vector.tensor_scalar(
                out=mrow, in0=lrow, scalar1=lmax, scalar2=None,
                op0=mybir.AluOpType.is_equal,
            )
            junk()
            mps = ps.tile([HALF, 512], F32, tag="outps", name="mps")
            nc.tensor.matmul(mps[:, 0:E], lhsT=ones1, rhs=mrow)
            mbc = small.tile([HALF, E], F32)
            nc.scalar.activation(
                out=mbc, in_=mps[:, 0:E],
                func=mybir.ActivationFunctionType.Copy,
            )
            junk()

            # ---- select expert weights (mask exactly one-hot) ----
            def sel_range(lo, hi):
                nc.vector.tensor_scalar(
                    out=usel[:, lo:hi], in0=u_all[:, 0, lo:hi],
                    scalar1=mbc[:, 0:1], scalar2=None,
                    op0=mybir.AluOpType.mult,
                )
                for e in range(1, E):
                    nc.vector.scalar_tensor_tensor(
                        out=usel[:, lo:hi], in0=u_all[:, e, lo:hi],
                        scalar=mbc[:, e : e + 1], in1=usel[:, lo:hi],
                        op0=mybir.AluOpType.mult, op1=mybir.AluOpType.add,
                    )

            # DVE: bias table, then Wh (gates first H matmuls), then wt2
            nc.vector.tensor_scalar(
                out=bsel, in0=bias_all[:, 0, :],
                scalar1=mbc[:, 0:1], scalar2=None,
                op0=mybir.AluOpType.mult,
            )
            junk()
            for e in range(1, E):
                nc.vector.scalar_tensor_tensor(
                    out=bsel, in0=bias_all[:, e, :],
                    scalar=mbc[:, e : e + 1], in1=bsel,
                    op0=mybir.AluOpType.mult, op1=mybir.AluOpType.add,
                )
            junk()
            sel_range(U_WH0, U_WH1 + QUARTER)      # [256:384)
            junk(rhs=usel[:, U_WH0 : U_WH0 + HALF], nf=HALF)
            wt2c = small.tile([HALF, 1], F32)
            nc.vector.tensor_scalar(
                out=wt2c, in0=u_all[:, 0, U_WT2 : U_WT2 + 1],
                scalar1=mbc[:, 0:1], scalar2=None, op0=mybir.AluOpType.mult,
            )
            for e in range(1, E):
                nc.vector.scalar_tensor_tensor(
                    out=wt2c, in0=u_all[:, e, U_WT2 : U_WT2 + 1],
                    scalar=mbc[:, e : e + 1], in1=wt2c,
                    op0=mybir.AluOpType.mult, op1=mybir.AluOpType.add,
                )
            nc.vector.tensor_scalar(
                out=usel[:, U_WT2 : U_WT2 + HALF], in0=ones128,
                scalar1=wt2c, scalar2=None, op0=mybir.AluOpType.mult,
            )
            junk(rhs=usel[:, U_WT2 : U_WT2 + HALF], nf=HALF)

            # Act computes the F-range partial products; DVE adds them.
            utmp = []
            for e in range(E):
                ut = sel.tile([HALF, 256], BF16, tag=f"ut{e}", name="ut")
                nc.scalar.activation(
                    out=ut, in_=u_all[:, e, U_F0 : U_F0 + 256],
                    func=mybir.ActivationFunctionType.Copy,
                    scale=mbc[:, e : e + 1],
                )
                utmp.append(ut)
            nc.vector.tensor_add(usel[:, 0:256], utmp[0], utmp[1])
            junk()
            for e in range(2, E):
                nc.vector.tensor_add(usel[:, 0:256], usel[:, 0:256], utmp[e])
            junk(rhs=usel[:, U_F0 : U_F0 + HALF], nf=HALF)
            junk(rhs=usel[:, U_F1 : U_F1 + HALF], nf=HALF)

            # rank-1 bias rows (needed only by F matmuls, FSKEW iters later)
            nc.vector.tensor_scalar(
                out=btr[0:1, :, :], in0=btr_all[0:1, 0, :, :],
                scalar1=mbc[0:1, 0:1], scalar2=None,
                op0=mybir.AluOpType.mult,
            )
            for e in range(1, E):
                nc.vector.scalar_tensor_tensor(
                    out=btr[0:1, :, :], in0=btr_all[0:1, e, :, :],
                    scalar=mbc[0:1, e : e + 1], in1=btr[0:1, :, :],
                    op0=mybir.AluOpType.mult, op1=mybir.AluOpType.add,
                )

            # ---- phase 2: pair-granular pipeline ----
            hsbl = [None] * NP
            ssbl = [None] * NP
            yl = [None] * NP

            for i in range(NP + FSKEW):
                if i < NP:
                    p = i
                    off, n0, n1 = PAIRS[p]
                    hps = ps.tile([HALF, 2, 512], F32, tag="hps2", name="hps")
                    for c, (co, cn) in enumerate([(off, n0), (off + n0, n1)]):
                        if cn == 0:
                            continue
                        nc.tensor.matmul(
                            hps[0:QUARTER, c, 0:cn],
                            lhsT=usel[:, U_WH0 : U_WH0 + QUARTER],
                            rhs=xb[:, 0, co : co + cn],
                        )
                        nc.tensor.matmul(
                            hps[QUARTER:HALF, c, 0:cn],
                            lhsT=usel[:, U_WH1 : U_WH1 + QUARTER],
                            rhs=xb[:, 1, co : co + cn],
                            tile_position=(0, QUARTER),
                        )
                    hsbl[p] = sb.tile([HALF, 2, 512], BF16, tag="hsb", name="hsb")
                    if n1:
                        nc.scalar.activation(
                            out=hsbl[p][:, :, :], in_=hps[:, :, :],
                            func=mybir.ActivationFunctionType.Relu,
                            bias=bsel[:, 0:1],
                        )
                    else:
                        nc.scalar.activation(
                            out=hsbl[p][:, 0, 0:n0], in_=hps[:, 0, 0:n0],
                            func=mybir.ActivationFunctionType.Relu,
                            bias=bsel[:, 0:1],
                        )
                if 0 <= i - 1 < NP:
                    p = i - 1
                    off, n0, n1 = PAIRS[p]
                    # aps4 bank layout (h-major): [a0(c0), a0(c1), a1(c0), a1(c1)]
                    aps4 = ps.tile([HALF, 4, 512], F32, tag="aps4", name="aps4")
                    for c, cn in enumerate([n0, n1]):
                        if cn == 0:
                            continue
                        nc.tensor.matmul(
                            aps4[:, 0 + c, 0:cn],
                            lhsT=usel[0:QUARTER, U_WT2 : U_WT2 + HALF],
                            rhs=hsbl[p][0:QUARTER, c, 0:cn],
                            tile_position=(0, 0),
                        )
                        nc.tensor.matmul(
                            aps4[:, 2 + c, 0:cn],
                            lhsT=usel[QUARTER:HALF, U_WT2 : U_WT2 + HALF],
                            rhs=hsbl[p][QUARTER:HALF, c, 0:cn],
                            tile_position=(QUARTER, 0),
                        )
                    ssbl[p] = sb.tile([HALF, 4, 512], BF16, tag="ssb", name="ssb")
                    if n1:
                        nc.scalar.activation(
                            out=ssbl[p][:, :, :], in_=aps4[:, :, :],
                            func=mybir.ActivationFunctionType.Sigmoid,
                            bias=bsel[:, 1:2],
                        )
                    else:
                        nc.scalar.activation(
                            out=ssbl[p][:, 0, 0:n0], in_=aps4[:, 0, 0:n0],
                            func=mybir.ActivationFunctionType.Sigmoid,
                            bias=bsel[:, 1:2],
                        )
                        nc.scalar.activation(
                            out=ssbl[p][:, 2, 0:n0], in_=aps4[:, 2, 0:n0],
                            func=mybir.ActivationFunctionType.Sigmoid,
                            bias=bsel[:, 1:2],
                        )
                    # y_h = x_h * s_h (bf16); ssb (h,c,j) flat matches xb cols
                    yl[p] = sb.tile([HALF, 2, 1024], BF16, tag="y", name="y")
                    nt = n0 + n1
                    if n1:
                        nc.vector.tensor_mul(
                            yl[p][:, :, :], xb[:, :, off : off + nt],
                            ssbl[p][:, :, :],
                        )
                    else:
                        nc.vector.tensor_mul(
                            yl[p][:, 0, 0:n0], xb[:, 0, off : off + n0],
                            ssbl[p][:, 0, 0:n0],
                        )
                        nc.vector.tensor_mul(
                            yl[p][:, 1, 0:n0], xb[:, 1, off : off + n0],
                            ssbl[p][:, 2, 0:n0],
                        )
                if 0 <= i - FSKEW < NP:
                    p = i - FSKEW
                    off, n0, n1 = PAIRS[p]
                    ops2 = ps.tile([HALF, 2, 512], F32, tag="outps", name="ops2")
                    for c, cn in enumerate([n0, n1]):
                        if cn == 0:
                            continue
                        nc.tensor.matmul(
                            ops2[:, c, 0:cn], lhsT=usel[:, U_F0 : U_F0 + HALF],
                            rhs=yl[p][:, 0, c * 512 : c * 512 + cn],
                            start=True, stop=False,
                        )
                        nc.tensor.matmul(
                            ops2[:, c, 0:cn], lhsT=usel[:, U_F1 : U_F1 + HALF],
                            rhs=yl[p][:, 1, c * 512 : c * 512 + cn],
                            start=False, stop=False,
                        )
                        nc.tensor.matmul(
                            ops2[:, c, 0:cn], lhsT=btr[0:1, 0, :],
                            rhs=ssbl[p][0:1, 0 + c, 0:cn],
                            start=False, stop=False,
                        )
                        nc.tensor.matmul(
                            ops2[:, c, 0:cn], lhsT=btr[0:1, 1, :],
                            rhs=ssbl[p][0:1, 2 + c, 0:cn],
                            start=False, stop=True,
                        )
                    nt = n0 + n1
                    nc.vector.tensor_copy(
                        osb[:, off : off + nt],
                        ops2[:, :, :] if n1 else ops2[:, 0, 0:n0],
                    )
                    nc.sync.dma_start(
                        out=out_d[:, off : off + nt], in_=osb[:, off : off + nt]
                    )

    nc.compile()
    return nc


def _pack_inputs(x, Wg, bg, Wrgb, brgb, Wtir, btir, Wt1, bt1, Wt2, bt2):
    import ml_dtypes

    eye = np.eye(HALF, dtype=np.float64)
    u = np.zeros((E, HALF, UF), dtype=np.float64)
    biasT = np.zeros((E, HALF, NBIAS), dtype=np.float64)
    btr = np.zeros((1, E, 2, HALF), dtype=np.float64)
    for e in range(E):
        F0 = eye + Wrgb[e].astype(np.float64)
        F1 = eye + Wtir[e].astype(np.float64)
        Wt1e = Wt1[e].astype(np.float64)
        u[e, :, U_F0 : U_F0 + HALF] = F0.T
        u[e, :, U_F1 : U_F1 + HALF] = F1.T
        u[e, :, U_WH0 : U_WH0 + QUARTER] = (Wt1e @ F0).T
        u[e, :, U_WH1 : U_WH1 + QUARTER] = (Wt1e @ F1).T
        u[e, :, U_WT2] = np.tile(Wt2[e, 0].astype(np.float64), 2)
        biasT[e, 0:QUARTER, 0] = Wt1e @ brgb[e].astype(np.float64) + bt1[e]
        biasT[e, QUARTER:HALF, 0] = Wt1e @ btir[e].astype(np.float64) + bt1[e]
        biasT[e, :, 1] = bt2[e, 0]
        btr[0, e, 0, :] = brgb[e]
        btr[0, e, 1, :] = btir[e]
    u = np.ascontiguousarray(u.transpose(1, 0, 2)).astype(ml_dtypes.bfloat16)
    bias = np.ascontiguousarray(biasT.transpose(1, 0, 2)).astype(np.float32)
    btr = btr.astype(ml_dtypes.bfloat16)

    wgt = Wg.T.astype(np.float32)                   # [256, 5]
    wg_p = np.ascontiguousarray(
        np.stack([wgt[:HALF], wgt[HALF:]], axis=1)
    ).astype(ml_dtypes.bfloat16)                    # [128, 2, 5]
    bgx = np.ascontiguousarray((bg * float(HW))[None, :].astype(np.float32))

    xp = np.ascontiguousarray(
        x.reshape(B, 2, HALF, HW).transpose(0, 2, 1, 3)
    ).astype(ml_dtypes.bfloat16)                    # [B, 128, 2, HW]

    common = {"u": u, "bias": bias, "btr": btr, "wg": wg_p, "bg": bgx}
    in_maps = []
    for b in range(B):
        m = dict(common)
        m["x"] = xp[b]
        in_maps.append(m)
    return in_maps


_NC_CACHE = {}


def _get_nc():
    if "nc" not in _NC_CACHE:
        _NC_CACHE["nc"] = build_nc()
    return _NC_CACHE["nc"]


def kernel(x, Wg, bg, Wrgb, brgb, Wtir, btir, Wt1, bt1, Wt2, bt2, **run_kw):
    nc = _get_nc()
    in_maps = _pack_inputs(
        np.asarray(x), np.asarray(Wg), np.asarray(bg), np.asarray(Wrgb),
        np.asarray(brgb), np.asarray(Wtir), np.asarray(btir),
        np.asarray(Wt1), np.asarray(bt1), np.asarray(Wt2), np.asarray(bt2),
    )
    res = run_bass_kernel_spmd(nc, in_maps, core_ids=list(range(NCORES)), **run_kw)
    out = np.stack(
        [np.asarray(r["out"]).astype(np.float32) for r in res.results], axis=0
    )
    if run_kw:
        kernel.last_results = res
    return out.reshape(B, HALF, H, W)
